# revision 1
# baseline (speedup 1.0000x reference)
"""Self-contained trn2 kernel for nn_DeepFusedGPTTransformerLayer.

Token-parallel fused GPT layer on 8 NeuronCores; see tk_kernel docstring.
The harness calls kernel(**inputs) with the FULL inputs; sharding, weight
folding, compile and gather happen here. Compiled graph is cached per
process.
"""
import sys
if '/opt/trn_rl_repo' not in sys.path:
    sys.path.insert(0, '/opt/trn_rl_repo')

_CACHE = {}


def kernel(**inputs):
    import tk_kernel as tk
    c = tk.Cfg()
    if "nc" not in _CACHE:
        _CACHE["nc"] = tk.build(c)
    out, _ = tk.run(_CACHE["nc"], c, inputs, trace=False)
    return out
